# revision 42
# baseline (speedup 1.0000x reference)
"""BiDAF attention-flow kernel for Trainium2 (8 NeuronCores, data-parallel).

Self-contained: hardcodes shapes B,C,Q,H2 = 64,512,64,256; n_labels=2.
kernel(**inputs) takes full unsharded inputs, shards batch over 8 cores,
runs one SPMD Bass/Tile kernel, gathers [8,2] per core -> [64,2].

Per-core math (8 examples, bf16 compute, fp32 accumulation):
  S = c @ diag(w_m) @ q^T + (c@w_c)[:,None] + (q@w_q)[None,:]
    - the c@w_c term folds into the matmul rhs (rhs = w_m*q^T + w_c),
    - the q@w_q term rides in via a K=1 all-ones broadcast matmul.
  P = exp(S) unstabilized (|S| is O(1) for this distribution), so
  row-softmax needs only row-sums, and b_att = softmax(max_j S) is just
  Pmax/sum(Pmax) with Pmax = max_j P  (exp is monotone).
  All transposes go through the PE (is_transpose matmuls); max-pools run
  in d-major layout as 2x-mode tensor_tensor max folds + short reduces;
  the c*q2c piece uses max(q2c*cmax, q2c*cmin) so it needs no extra pass.

Structure (v2, ~66us vs 71.6us baseline): identity first on the gpsimd
queue (iota would otherwise stall every transpose behind load
descriptor-gen), loads issued unchained in pipeline order (q-lower,
e0, e1, pair1, q-upper, pair2, pair3 -- FIFO descriptor drain gives
in-order chunk arrival at full read bandwidth); q-side prep hoisted
into one phase (q^T, rhs_qm via DVE tensor_scalar 4x, qw rows); the
per-pair work is split into stages A (c^T transposes+evac, S matmuls,
exp), D (row sums/maxes, 1/den, Pn -- pure DVE) and E (Pn^T, b_att,
c2q, fold chains, q2c, piece3), emitted stage-skewed
  A0; A1 D0; A2 D1 E0; A3 D2 E1; D3 E2; E3
so every engine's in-order queue sees ready work from older pairs
ahead of blocked work from newer ones.  All PSUM->SBUF evacuations
ride the ACT engine; pieces {c2q, c*c2q, max_c c} share one stacked
all-max fold pyramid (feature chunks permuted, wlab re-indexed to
match); DVE fold work (~35us/core) is the kernel's critical path.
"""

import os
import sys

for _p in ("/opt/trn_rl_repo", "/opt/pypackages"):
    if os.path.isdir(_p) and _p not in sys.path:
        sys.path.insert(0, _p)

import numpy as np

import concourse.bass as bass
import concourse.bacc as bacc
import concourse.tile as tile
import concourse.mybir as mybir
from concourse.bass_utils import run_bass_kernel_spmd
from concourse.masks import make_identity
from concourse.tile_rust import add_dep_helper

F32 = mybir.dt.float32
BF16 = mybir.dt.bfloat16
AX = mybir.AxisListType
OP = mybir.AluOpType
AF = mybir.ActivationFunctionType

N_CORES = 8
B, C, Q, H2 = 64, 512, 64, 256
NL = 2
EX = B // N_CORES          # examples per core = 8
CH = C // 128              # context chunks of 128 = 4
DH = H2 // 128             # feature chunks of 128 = 2
NK = 4 * DH                # final feature chunks (4 pieces x DH) = 8


def _body(tc, ctx, fd, fq, wsim, wlab, blab, out):
    nc = tc.nc

    consts = ctx.enter_context(tc.tile_pool(name="consts", bufs=1))
    bigbuf = ctx.enter_context(tc.tile_pool(name="bigbuf", bufs=1))
    den_pool = ctx.enter_context(tc.tile_pool(name="den", bufs=3))
    scr_pool = ctx.enter_context(tc.tile_pool(name="scr", bufs=6))
    q2_pool = ctx.enter_context(tc.tile_pool(name="q2", bufs=3))
    sb_small = ctx.enter_context(tc.tile_pool(name="small", bufs=1))

    ps_tp_pool = ctx.enter_context(tc.tile_pool(name="ptp", bufs=2, space="PSUM"))
    ps_s_pool = ctx.enter_context(tc.tile_pool(name="pss", bufs=2, space="PSUM"))
    ps_c2q_pool = ctx.enter_context(tc.tile_pool(name="psc", bufs=2, space="PSUM"))
    ps_misc_pool = ctx.enter_context(tc.tile_pool(name="psm", bufs=2, space="PSUM"))

    # ---- identity FIRST on the gpsimd queue (iota/affine_select live
    # there); anything queued after the load descriptor-gens would stall
    # every PE transpose behind ~5us of descriptor generation. ----
    id_bf = consts.tile([128, 128], BF16)
    make_identity(nc, id_bf[:, :])
    id_f32 = consts.tile([64, 64], F32)
    make_identity(nc, id_f32[:, :])

    # ---- big inputs: cast-load fp32 -> bf16 (SWDGE), unchained.
    # Pair-0 chunk first (it gates the compute pipeline); q lower half
    # next (q^T prep); the rest in pipeline order.  Descriptors drain in
    # FIFO order per queue so chunk k completes right after chunk k-1 at
    # full read bandwidth. ----
    q_dup = bigbuf.tile([128, EX, H2], BF16)        # q on both 64-partition halves
    c_nat = bigbuf.tile([128, EX, CH, H2], BF16)    # partition = c%128 (p ch order)

    def load_c(lo, hi):
        nc.gpsimd.dma_start(
            c_nat[:, lo:hi, :, :],
            fd[lo:hi, :, :].rearrange("e (p ch) d -> p e ch d", p=128),
        )

    nc.gpsimd.dma_start(q_dup[0:64, :, :], fq[:, :, :].rearrange("e j d -> j e d"))
    load_c(0, 1)
    load_c(1, 2)
    load_c(2, 4)
    nc.gpsimd.dma_start(q_dup[64:128, :, :], fq[:, :, :].rearrange("e j d -> j e d"))
    load_c(4, 6)
    load_c(6, 8)

    # ---- early fp32 q copy (HWDGE, sync queue): lands ~5us before the
    # SWDGE bf16 copy, pulling the q^T/rhs_qm prep chain off the
    # critical path.  Extra 0.5MB of HBM traffic in an otherwise idle
    # DMA window. ----
    w_sb = consts.tile([128, 6], F32)          # col = t*2+dh; t: 0=w_c 1=w_q 2=w_m
    nc.sync.dma_start(w_sb[:, :], wsim[:].rearrange("(t dh p) -> p (t dh)", dh=DH, p=128))
    q_f32 = bigbuf.tile([64, EX, H2], F32)
    nc.sync.dma_start(q_f32[:, :, :], fq[:, :, :].rearrange("e j d -> j e d"))

    # ---- remaining constants / weights (HWDGE, sync queue) ----
    wq_bf = consts.tile([128, DH], BF16)       # w_q as bf16 matmul operand
    nc.vector.tensor_copy(wq_bf[:, :], w_sb[:, 2:4])
    wlab_sb = consts.tile([128, NK, NL], F32)  # chunk k = piece*DH+dh
    nc.sync.dma_start(wlab_sb[:, :, :], wlab[:, :].rearrange("(k p) l -> p k l", p=128))
    b_sb = consts.tile([1, NL], F32)
    nc.sync.dma_start(b_sb[0:1, :], blab[:].rearrange("(o l) -> o l", o=1))
    ones_bf = consts.tile([1, 128], BF16)      # K=1 broadcast lhsT
    nc.vector.memset(ones_bf[0:1, :], 1.0)
    ones128_bf = consts.tile([128, 1], BF16)   # partition-sum lhsT
    nc.vector.memset(ones128_bf[:, :], 1.0)
    ones_f32 = consts.tile([1, 128], F32)      # broadcast lhsT + [1,1] identity
    nc.vector.memset(ones_f32[0:1, :], 1.0)

    # HAM warmup: dep-free matmuls keep the PE busy until pair-0 data
    # lands so the clock gate is at 8/8 when the real matmuls arrive.
    ps_warm = ps_misc_pool.tile([128, 512], F32, tag="misc")
    N_WARM = 2
    for r in range(N_WARM):
        nc.tensor.matmul(
            ps_warm[0:64, 0:64], id_bf[:, 0:64], id_bf[:, 64:128],
            start=(r == 0), stop=(r == N_WARM - 1), skip_group_check=True,
        )

    def psum_copy(dst_ap, src_ap):
        """PSUM->SBUF evacuations ride the ACT engine (DVE is loaded)."""
        nc.scalar.copy(dst_ap, src_ap)

    def pe_transpose_group(psum_view, srcs):
        """Transpose each [128|64,128] src into psum_view[:, k, :] via PE."""
        first = None
        for k, src in enumerate(srcs):
            mm = nc.tensor.matmul(
                psum_view[:, k, :], src, id_bf[0:src.shape[0], 0:src.shape[0]],
                is_transpose=True,
                start=(first is None), stop=(k == len(srcs) - 1),
                skip_group_check=True,
            )
            if first is None:
                first = mm
            else:
                add_dep_helper(mm.ins, first.ins, sync=False, reason="bank order")
        return first

    # ---- persistent SBUF tensors ----
    c_T = bigbuf.tile([128, EX, DH, C], BF16)       # [d', e, dh, c]
    qT_sb = bigbuf.tile([128, EX, DH, Q], BF16)     # [d', e, dh, j]
    rhs_qm = bigbuf.tile([128, EX, DH, Q], BF16)    # w_m*q^T + w_c
    qwrow = sb_small.tile([1, EX, Q], BF16)         # (q @ w_q) rows
    P_all = sb_small.tile([128, CH, EX, Q], BF16)
    Pn_all = sb_small.tile([128, CH, EX, Q], BF16)
    PT_all = sb_small.tile([128, EX // 2, CH, 128], BF16)
    c2q_sb = bigbuf.tile([128, EX, DH, C], BF16)    # c2q^T (d-major, normalized)
    pm_col = sb_small.tile([128, CH, EX], BF16)     # Pmax (b_att numerators)
    final_f = sb_small.tile([128, NK * EX], F32)    # col = (piece*DH+dh)*EX + e
    cmin_f = sb_small.tile([128, DH * EX], F32)     # col = dh*EX + e
    r_sb = sb_small.tile([128, EX], F32)            # 1/sum(pm), bcast over partitions
    sumb = sb_small.tile([1, EX], F32)
    recipb = sb_small.tile([1, EX], F32)
    out_sb = sb_small.tile([EX, NL], F32)
    q2cT_sb = sb_small.tile([128, EX, DH], F32)

    fview = final_f[:, :].rearrange("p (pc dh e) -> p pc dh e", pc=4, dh=DH)
    cminv = cmin_f[:, :].rearrange("p (dh e) -> p dh e", dh=DH)

    # ---------- phase Q (once): q^T, rhs_qm, qw rows ----------
    # 16 q^T transposes in 2 groups of 8, evac via ACT; rhs_qm via DVE
    # tensor_scalar (4x mode, per-partition scale/bias); qw via 2
    # accumulating matmuls over the full 8-example q^T.
    for g in range(2):
        tp_flat = ps_s_pool.tile([128, CH, 2, Q], F32, tag="ps_s")
        tp = tp_flat[:, :, :, :].rearrange("p c s j -> p (c s) j")
        first = None
        for k, (e, dh) in enumerate([(e, dh) for e in range(4 * g, 4 * g + 4)
                                     for dh in range(DH)]):
            mm = nc.tensor.matmul(
                tp[:, k, :], q_f32[:, e, dh * 128:(dh + 1) * 128], id_f32[:, :],
                is_transpose=True,
                start=(first is None), stop=(k == 7), skip_group_check=True,
            )
            if first is None:
                first = mm
            else:
                add_dep_helper(mm.ins, first.ins, sync=False, reason="bank order")
        psum_copy(
            qT_sb[:, 4 * g:4 * g + 4, :, :].rearrange("p e dh j -> p (e dh) j"),
            tp[:, :, :])
    for dh in range(DH):
        nc.vector.tensor_scalar(
            rhs_qm[:, :, dh, :], qT_sb[:, :, dh, :],
            w_sb[:, 4 + dh:5 + dh], w_sb[:, 0 + dh:1 + dh],
            op0=OP.mult, op1=OP.add,
        )
    ps_qw = ps_misc_pool.tile([128, 512], F32, tag="misc")
    for dh in range(DH):
        nc.tensor.matmul(
            ps_qw[0:1, 0:EX * Q].rearrange("o (e j) -> o e j", e=EX),
            wq_bf[:, dh:dh + 1],
            qT_sb[:, :, dh, :],
            start=(dh == 0), stop=(dh == DH - 1),
        )
    nc.scalar.copy(qwrow[0:1, :, :], ps_qw[0:1, 0:EX * Q].rearrange("o (e j) -> o e j", e=EX))

    # ---------- per-pair pipeline, stage-skewed ----------
    # Engine queues execute in program order, so pair p's late stages must
    # not sit ahead of pair p+1's independent early stages.  Emit rounds:
    # A(0); A(1) D(0); A(2) D(1) E(0); A(3) D(2) E(1); D(3) E(2); E(3).

    def stage_A(pair):
        """c^T transposes+evac, S matmuls, exp."""
        e0, e1 = 2 * pair, 2 * pair + 1
        eP = slice(e0, e1 + 1)
        for e in (e0, e1):
            for dh in range(DH):
                tp2 = ps_tp_pool.tile([128, CH, 128], BF16, tag="tp")
                pe_transpose_group(
                    tp2,
                    [c_nat[:, e, chk, dh * 128:(dh + 1) * 128] for chk in range(CH)],
                )
                psum_copy(c_T[:, e, dh, :], tp2[:, :, :])

        ps_s = ps_s_pool.tile([128, CH, 2, Q], F32)
        first_mm = None
        for slot in range(2):
            e = e0 + slot
            for chk in range(CH):
                for dh in range(DH):
                    mm = nc.tensor.matmul(
                        ps_s[:, chk, slot, :],
                        c_T[:, e, dh, chk * 128:(chk + 1) * 128],
                        rhs_qm[:, e, dh, :],
                        start=(first_mm is None), stop=False,
                        skip_group_check=True,
                    )
                    if first_mm is None:
                        first_mm = mm
                    else:
                        add_dep_helper(mm.ins, first_mm.ins, sync=False,
                                       reason="bank clear order")
            mm = nc.tensor.matmul(
                ps_s[:, :, slot, :],
                ones_bf[0:1, :],
                qwrow[0:1, e, :].unsqueeze(1).broadcast_to([1, CH, Q]),
                start=False, stop=(slot == 1),
                skip_group_check=True,
            )
            add_dep_helper(mm.ins, first_mm.ins, sync=False, reason="bank clear order")

        nc.scalar.activation(P_all[:, :, eP, :], ps_s[:, :, :, :], AF.Exp)

    def stage_D(pair):
        """Row sums + maxes, reciprocal, Pn (all DVE)."""
        e0, e1 = 2 * pair, 2 * pair + 1
        eP = slice(e0, e1 + 1)
        pview = P_all[:, :, eP, :]
        den = den_pool.tile([128, CH, 2], F32)
        nc.vector.reduce_sum(den[:, :, :], pview, axis=AX.X)
        nc.vector.tensor_reduce(pm_col[:, :, eP], pview, axis=AX.X, op=OP.max)
        rden = den_pool.tile([128, CH, 2], F32, tag="rden")
        nc.vector.reciprocal(rden[:, :, :], den[:, :, :])
        nc.vector.tensor_tensor(
            Pn_all[:, :, eP, :], pview,
            rden[:, :, :].unsqueeze(3).broadcast_to([128, CH, 2, Q]),
            op=OP.mult,
        )

    def stage_E(pair):
        """Pn^T, b_att norm, c2q, folds, q2c, piece3."""
        e0, e1 = 2 * pair, 2 * pair + 1
        eP = slice(e0, e1 + 1)

        tp = ps_tp_pool.tile([128, CH, 128], BF16, tag="tp")
        pe_transpose_group(tp, [Pn_all[:, chk, eP, :] for chk in range(CH)])
        psum_copy(PT_all[:, pair, :, :], tp[:, :, :])

        # b_att normalizers + q2c (PE-light, pulls work off the tail)
        ps_pair = ps_misc_pool.tile([128, 512], F32, tag="misc")
        nc.tensor.matmul(
            ps_pair[0:1, 0:2 * CH].rearrange("o (e c) -> o e c", c=CH),
            ones128_bf[:, :],
            pm_col[:, :, eP].rearrange("p c e -> p e c"),
            start=True, stop=True,
        )
        nc.vector.reduce_sum(
            sumb[0:1, eP],
            ps_pair[0:1, 0:2 * CH].rearrange("o (e c) -> o e c", c=CH),
            axis=AX.X,
        )
        nc.vector.reciprocal(recipb[0:1, eP], sumb[0:1, eP])
        nc.tensor.matmul(
            ps_pair[:, 8:10], ones_f32[0:1, :], recipb[0:1, eP],
            start=True, stop=True,
        )
        nc.scalar.copy(r_sb[:, eP], ps_pair[:, 8:10])

        for dh in range(DH):
            for slot in range(2):
                e = e0 + slot
                ps_c2q = ps_c2q_pool.tile([128, C], F32)
                nc.tensor.matmul(
                    ps_c2q[:, :],
                    q_dup[slot * 64:slot * 64 + 64, e, dh * 128:(dh + 1) * 128],
                    PT_all[slot * 64:slot * 64 + 64, pair, :, :],
                    start=True, stop=True,
                    tile_position=(slot * 64, 0),
                )
                psum_copy(c2q_sb[:, e, dh, :], ps_c2q[:, :])

        cT_p = c_T[:, eP, :, :]                    # [128, 2, DH, C]
        c2_p = c2q_sb[:, eP, :, :]
        prod = scr_pool.tile([128, 2, DH, C], BF16, tag="prod")
        # per-dh split so folding starts after half the c2q evacuations
        stk1 = scr_pool.tile([128, 3, 2, DH, 256], BF16, tag="stk1")
        for dh in range(DH):
            nc.vector.tensor_tensor(
                prod[:, :, dh, :], cT_p[:, :, dh, :], c2_p[:, :, dh, :], op=OP.mult)
            nc.vector.tensor_tensor(
                stk1[:, 0, :, dh, :], c2_p[:, :, dh, 0:256], c2_p[:, :, dh, 256:512], op=OP.max)
            nc.vector.tensor_tensor(
                stk1[:, 1, :, dh, :], prod[:, :, dh, 0:256], prod[:, :, dh, 256:512], op=OP.max)
            nc.vector.tensor_tensor(
                stk1[:, 2, :, dh, :], cT_p[:, :, dh, 0:256], cT_p[:, :, dh, 256:512], op=OP.max)
        stk2 = scr_pool.tile([128, 3, 2, DH, 128], BF16, tag="stk2")
        nc.vector.tensor_tensor(
            stk2[:, :, :, :, :], stk1[:, :, :, :, 0:128], stk1[:, :, :, :, 128:256], op=OP.max)
        stk3 = scr_pool.tile([128, 3, 2, DH, 64], BF16, tag="stk3")
        nc.vector.tensor_tensor(
            stk3[:, :, :, :, :], stk2[:, :, :, :, 0:64], stk2[:, :, :, :, 64:128], op=OP.max)
        stk4 = scr_pool.tile([128, 3, 2, DH, 32], BF16, tag="stk4")
        nc.vector.tensor_tensor(
            stk4[:, :, :, :, :], stk3[:, :, :, :, 0:32], stk3[:, :, :, :, 32:64], op=OP.max)
        nc.vector.tensor_reduce(
            fview[:, 0:3, :, eP].rearrange("p pc dh e -> p pc e dh"),
            stk4[:, :, :, :, :], axis=AX.X, op=OP.max)

        f1 = scr_pool.tile([128, 2, DH, 256], BF16, tag="bigA")
        nc.vector.tensor_tensor(
            f1[:, :, :, :], cT_p[:, :, :, 0:256], cT_p[:, :, :, 256:512], op=OP.min)
        f2 = scr_pool.tile([128, 2, DH, 128], BF16, tag="bigB")
        nc.vector.tensor_tensor(
            f2[:, :, :, :], f1[:, :, :, 0:128], f1[:, :, :, 128:256], op=OP.min)
        f3 = scr_pool.tile([128, 2, DH, 64], BF16, tag="bigC")
        nc.vector.tensor_tensor(
            f3[:, :, :, :], f2[:, :, :, 0:64], f2[:, :, :, 64:128], op=OP.min)
        nc.vector.tensor_reduce(
            cminv[:, :, eP].rearrange("p dh e -> p e dh"),
            f3[:, :, :, :], axis=AX.X, op=OP.min)

        for slot in range(2):
            e = e0 + slot
            ps_m2 = ps_misc_pool.tile([128, 512], F32, tag="misc")
            for chk in range(CH):
                nc.tensor.matmul(
                    ps_m2[0:1, 64:64 + H2],
                    pm_col[:, chk, e:e + 1],
                    c_nat[:, e, chk, :],
                    start=(chk == 0), stop=(chk == CH - 1),
                )
            q2c_sc = q2_pool.tile([1, H2], F32)
            nc.scalar.mul(q2c_sc[0:1, :], ps_m2[0:1, 64:64 + H2], r_sb[0:1, e:e + 1])
            for dh in range(DH):
                nc.tensor.matmul(
                    ps_m2[:, 320 + dh:321 + dh],
                    q2c_sc[0:1, dh * 128:(dh + 1) * 128],
                    ones_f32[0:1, 0:1],
                    is_transpose=True,
                    start=(dh == 0), stop=(dh == DH - 1),
                    skip_group_check=True,
                )
            nc.scalar.copy(q2cT_sb[:, e, :], ps_m2[:, 320:322])

        s3a = scr_pool.tile([128, 2, DH], F32, tag="s3a")
        s3b = scr_pool.tile([128, 2, DH], F32, tag="s3b")
        nc.vector.tensor_tensor(
            s3a[:, :, :], q2cT_sb[:, eP, :],
            fview[:, 2, :, eP].rearrange("p dh e -> p e dh"), op=OP.mult)
        nc.vector.tensor_tensor(
            s3b[:, :, :], q2cT_sb[:, eP, :],
            cminv[:, :, eP].rearrange("p dh e -> p e dh"), op=OP.mult)
        nc.vector.tensor_tensor(
            fview[:, 3, :, eP].rearrange("p dh e -> p e dh"),
            s3a[:, :, :], s3b[:, :, :], op=OP.max)

    stage_A(0)
    stage_A(1); stage_D(0)
    stage_A(2); stage_D(1); stage_E(0)
    stage_A(3); stage_D(2); stage_E(1)
    stage_D(3); stage_E(2)
    stage_E(3)

    # ---------- final: out = max-pooled features @ w_label + b ----------
    ps_out = ps_misc_pool.tile([128, 512], F32, tag="misc")
    REF_PC = (1, 2, 0, 3)
    for k in range(NK):
        pc, dh = k // DH, k % DH
        nc.tensor.matmul(
            ps_out[0:EX, 0:NL], final_f[:, k * EX:(k + 1) * EX],
            wlab_sb[:, REF_PC[pc] * DH + dh, :],
            start=(k == 0), stop=False, skip_group_check=True,
        )
    nc.tensor.matmul(
        ps_out[0:EX, 0:NL], ones_f32[0:1, 0:EX], b_sb[0:1, :],
        start=False, stop=True, skip_group_check=True,
    )
    nc.scalar.copy(out_sb[:, :], ps_out[0:EX, 0:NL])
    nc.sync.dma_start(out[:, :], out_sb[:, :])


def build_nc():
    nc = bacc.Bacc("TRN2", target_bir_lowering=False, debug=False)
    fd = nc.dram_tensor("fd", [EX, C, H2], F32, kind="ExternalInput")
    fq = nc.dram_tensor("fq", [EX, Q, H2], F32, kind="ExternalInput")
    wsim = nc.dram_tensor("wsim", [3 * H2], F32, kind="ExternalInput")
    wlab = nc.dram_tensor("wlab", [4 * H2, NL], F32, kind="ExternalInput")
    blab = nc.dram_tensor("blab", [NL], F32, kind="ExternalInput")
    out = nc.dram_tensor("out", [EX, NL], F32, kind="ExternalOutput")

    from contextlib import ExitStack
    with tile.TileContext(nc) as tc:
        with ExitStack() as ctx:
            _body(tc, ctx, fd[:, :, :], fq[:, :, :], wsim[:], wlab[:, :], blab[:], out[:, :])
    nc.compile()
    return nc


_NC_CACHE = None


def run(inputs, trace=False):
    global _NC_CACHE
    if _NC_CACHE is None:
        _NC_CACHE = build_nc()
    nc = _NC_CACHE

    fd = np.ascontiguousarray(np.asarray(inputs["feature_document"], dtype=np.float32))
    fq = np.ascontiguousarray(np.asarray(inputs["feature_query"], dtype=np.float32))
    wsim = np.ascontiguousarray(np.asarray(inputs["w_sim"], dtype=np.float32))
    wlab = np.ascontiguousarray(np.asarray(inputs["w_label"], dtype=np.float32))
    blab = np.ascontiguousarray(np.asarray(inputs["b_label"], dtype=np.float32))

    in_maps = []
    for core in range(N_CORES):
        sl = slice(core * EX, (core + 1) * EX)
        in_maps.append({
            "fd": fd[sl], "fq": fq[sl],
            "wsim": wsim, "wlab": wlab, "blab": blab,
        })
    res = run_bass_kernel_spmd(nc, in_maps, list(range(N_CORES)), trace=trace)
    outs = np.concatenate([np.asarray(res.results[i]["out"]) for i in range(N_CORES)], axis=0)
    return outs.astype(np.float32), res


def kernel(**inputs):
    outs, _ = run(inputs, trace=False)
    return outs


# revision 43
# speedup vs baseline: 1.0554x; 1.0554x over previous
"""BiDAF attention-flow kernel for Trainium2 (8 NeuronCores, data-parallel).

Self-contained: hardcodes shapes B,C,Q,H2 = 64,512,64,256; n_labels=2.
kernel(**inputs) takes full unsharded inputs, shards batch over 8 cores,
runs one SPMD Bass/Tile kernel, gathers [8,2] per core -> [64,2].

Per-core math (8 examples, bf16 compute, fp32 accumulation):
  S = c @ diag(w_m) @ q^T + (c@w_c)[:,None] + (q@w_q)[None,:]
    - the c@w_c term folds into the matmul rhs (rhs = w_m*q^T + w_c),
    - the q@w_q term rides in via a K=1 all-ones broadcast matmul.
  P = exp(S) unstabilized (|S| is O(1) for this distribution), so
  row-softmax needs only row-sums, and b_att = softmax(max_j S) is just
  Pmax/sum(Pmax) with Pmax = max_j P  (exp is monotone).
  All transposes go through the PE (is_transpose matmuls); max-pools run
  in d-major layout as 2x-mode tensor_tensor max folds + short reduces;
  the c*q2c piece uses max(q2c*cmax, q2c*cmin) so it needs no extra pass.

Structure (v2, ~66us vs 71.6us baseline): identity first on the gpsimd
queue (iota would otherwise stall every transpose behind load
descriptor-gen), loads issued unchained in pipeline order (q-lower,
e0, e1, pair1, q-upper, pair2, pair3 -- FIFO descriptor drain gives
in-order chunk arrival at full read bandwidth); q-side prep hoisted
into one phase (q^T, rhs_qm via DVE tensor_scalar 4x, qw rows); the
per-pair work is split into stages A (c^T transposes+evac, S matmuls,
exp), D (row sums/maxes, 1/den, Pn -- pure DVE) and E (Pn^T, b_att,
c2q, fold chains, q2c, piece3), emitted stage-skewed
  A0; A1 D0; A2 D1 E0; A3 D2 E1; D3 E2; E3
so every engine's in-order queue sees ready work from older pairs
ahead of blocked work from newer ones.  All PSUM->SBUF evacuations
ride the ACT engine; pieces {c2q, c*c2q, max_c c} share one stacked
all-max fold pyramid (feature chunks permuted, wlab re-indexed to
match); DVE fold work (~35us/core) is the kernel's critical path.
"""

import os
import sys

for _p in ("/opt/trn_rl_repo", "/opt/pypackages"):
    if os.path.isdir(_p) and _p not in sys.path:
        sys.path.insert(0, _p)

import numpy as np

import concourse.bass as bass
import concourse.bacc as bacc
import concourse.tile as tile
import concourse.mybir as mybir
from concourse.bass_utils import run_bass_kernel_spmd
from concourse.masks import make_identity
from concourse.tile_rust import add_dep_helper

F32 = mybir.dt.float32
BF16 = mybir.dt.bfloat16
AX = mybir.AxisListType
OP = mybir.AluOpType
AF = mybir.ActivationFunctionType

N_CORES = 8
B, C, Q, H2 = 64, 512, 64, 256
NL = 2
EX = B // N_CORES          # examples per core = 8
CH = C // 128              # context chunks of 128 = 4
DH = H2 // 128             # feature chunks of 128 = 2
NK = 4 * DH                # final feature chunks (4 pieces x DH) = 8


def _body(tc, ctx, fd, fq, wsim, wlab, blab, out):
    nc = tc.nc

    consts = ctx.enter_context(tc.tile_pool(name="consts", bufs=1))
    bigbuf = ctx.enter_context(tc.tile_pool(name="bigbuf", bufs=1))
    den_pool = ctx.enter_context(tc.tile_pool(name="den", bufs=3))
    scr_pool = ctx.enter_context(tc.tile_pool(name="scr", bufs=6))
    q2_pool = ctx.enter_context(tc.tile_pool(name="q2", bufs=3))
    sb_small = ctx.enter_context(tc.tile_pool(name="small", bufs=1))

    ps_tp_pool = ctx.enter_context(tc.tile_pool(name="ptp", bufs=2, space="PSUM"))
    ps_s_pool = ctx.enter_context(tc.tile_pool(name="pss", bufs=2, space="PSUM"))
    ps_c2q_pool = ctx.enter_context(tc.tile_pool(name="psc", bufs=2, space="PSUM"))
    ps_misc_pool = ctx.enter_context(tc.tile_pool(name="psm", bufs=2, space="PSUM"))

    # ---- identity FIRST on the gpsimd queue (iota/affine_select live
    # there); anything queued after the load descriptor-gens would stall
    # every PE transpose behind ~5us of descriptor generation. ----
    id_bf = consts.tile([128, 128], BF16)
    make_identity(nc, id_bf[:, :])
    id_f32 = consts.tile([64, 64], F32)
    make_identity(nc, id_f32[:, :])

    # ---- big inputs: cast-load fp32 -> bf16 (SWDGE), unchained.
    # Pair-0 chunk first (it gates the compute pipeline); q lower half
    # next (q^T prep); the rest in pipeline order.  Descriptors drain in
    # FIFO order per queue so chunk k completes right after chunk k-1 at
    # full read bandwidth. ----
    q_dup = bigbuf.tile([128, EX, H2], BF16)        # q on both 64-partition halves
    c_nat = bigbuf.tile([128, EX, CH, H2], BF16)    # partition = c%128 (p ch order)

    def load_c(lo, hi):
        nc.gpsimd.dma_start(
            c_nat[:, lo:hi, :, :],
            fd[lo:hi, :, :].rearrange("e (p ch) d -> p e ch d", p=128),
        )

    nc.gpsimd.dma_start(q_dup[0:64, :, :], fq[:, :, :].rearrange("e j d -> j e d"))
    load_c(0, 1)
    load_c(1, 2)
    load_c(2, 4)
    nc.gpsimd.dma_start(q_dup[64:128, :, :], fq[:, :, :].rearrange("e j d -> j e d"))
    load_c(4, 6)
    load_c(6, 8)

    # ---- early fp32 q copy (HWDGE, sync queue): lands ~5us before the
    # SWDGE bf16 copy, pulling the q^T/rhs_qm prep chain off the
    # critical path.  Extra 0.5MB of HBM traffic in an otherwise idle
    # DMA window. ----
    q_f32 = bigbuf.tile([64, EX, H2], F32)
    nc.sync.dma_start(q_f32[:, :, :], fq[:, :, :].rearrange("e j d -> j e d"))

    # ---- constants / weights in SBUF (HWDGE, sync queue) ----
    w_sb = consts.tile([128, 6], F32)          # col = t*2+dh; t: 0=w_c 1=w_q 2=w_m
    nc.sync.dma_start(w_sb[:, :], wsim[:].rearrange("(t dh p) -> p (t dh)", dh=DH, p=128))
    wq_bf = consts.tile([128, DH], BF16)       # w_q as bf16 matmul operand
    nc.vector.tensor_copy(wq_bf[:, :], w_sb[:, 2:4])
    wlab_sb = consts.tile([128, NK, NL], F32)  # chunk k = piece*DH+dh
    nc.sync.dma_start(wlab_sb[:, :, :], wlab[:, :].rearrange("(k p) l -> p k l", p=128))
    b_sb = consts.tile([1, NL], F32)
    nc.sync.dma_start(b_sb[0:1, :], blab[:].rearrange("(o l) -> o l", o=1))
    ones_bf = consts.tile([1, 128], BF16)      # K=1 broadcast lhsT
    nc.vector.memset(ones_bf[0:1, :], 1.0)
    ones128_bf = consts.tile([128, 1], BF16)   # partition-sum lhsT
    nc.vector.memset(ones128_bf[:, :], 1.0)
    ones_f32 = consts.tile([1, 128], F32)      # broadcast lhsT + [1,1] identity
    nc.vector.memset(ones_f32[0:1, :], 1.0)

    # HAM warmup: dep-free matmuls keep the PE busy until pair-0 data
    # lands so the clock gate is at 8/8 when the real matmuls arrive.
    ps_warm = ps_misc_pool.tile([128, 512], F32, tag="misc")
    N_WARM = 2
    for r in range(N_WARM):
        nc.tensor.matmul(
            ps_warm[0:64, 0:64], id_bf[:, 0:64], id_bf[:, 64:128],
            start=(r == 0), stop=(r == N_WARM - 1), skip_group_check=True,
        )

    def psum_copy(dst_ap, src_ap):
        """PSUM->SBUF evacuations ride the ACT engine (DVE is loaded)."""
        nc.scalar.copy(dst_ap, src_ap)

    def pe_transpose_group(psum_view, srcs):
        """Transpose each [128|64,128] src into psum_view[:, k, :] via PE."""
        first = None
        for k, src in enumerate(srcs):
            mm = nc.tensor.matmul(
                psum_view[:, k, :], src, id_bf[0:src.shape[0], 0:src.shape[0]],
                is_transpose=True,
                start=(first is None), stop=(k == len(srcs) - 1),
                skip_group_check=True,
            )
            if first is None:
                first = mm
            else:
                add_dep_helper(mm.ins, first.ins, sync=False, reason="bank order")
        return first

    # ---- persistent SBUF tensors ----
    c_T = bigbuf.tile([128, EX, DH, C], BF16)       # [d', e, dh, c]
    qT_sb = bigbuf.tile([128, EX, DH, Q], BF16)     # [d', e, dh, j]
    rhs_qm = bigbuf.tile([128, EX, DH, Q], BF16)    # w_m*q^T + w_c
    qwrow = sb_small.tile([1, EX, Q], BF16)         # (q @ w_q) rows
    P_all = sb_small.tile([128, CH, EX, Q], BF16)
    Pn_all = sb_small.tile([128, CH, EX, Q], BF16)
    PT_all = sb_small.tile([128, EX // 2, CH, 128], BF16)
    c2q_sb = bigbuf.tile([128, EX, DH, C], BF16)    # c2q^T (d-major, normalized)
    pm_col = sb_small.tile([128, CH, EX], BF16)     # Pmax (b_att numerators)
    final_f = sb_small.tile([128, NK * EX], F32)    # col = (piece*DH+dh)*EX + e
    cmin_f = sb_small.tile([128, DH * EX], F32)     # col = dh*EX + e
    r_sb = sb_small.tile([128, EX], F32)            # 1/sum(pm), bcast over partitions
    sumb = sb_small.tile([1, EX], F32)
    recipb = sb_small.tile([1, EX], F32)
    out_sb = sb_small.tile([EX, NL], F32)
    q2cT_sb = sb_small.tile([128, EX, DH], F32)

    fview = final_f[:, :].rearrange("p (pc dh e) -> p pc dh e", pc=4, dh=DH)
    cminv = cmin_f[:, :].rearrange("p (dh e) -> p dh e", dh=DH)

    # ---------- phase Q (once): q^T, rhs_qm, qw rows ----------
    # 16 q^T transposes in 2 groups of 8, evac via ACT; rhs_qm via DVE
    # tensor_scalar (4x mode, per-partition scale/bias); qw via 2
    # accumulating matmuls over the full 8-example q^T.
    for g in range(2):
        tp_flat = ps_s_pool.tile([128, CH, 2, Q], F32, tag="ps_s")
        tp = tp_flat[:, :, :, :].rearrange("p c s j -> p (c s) j")
        first = None
        for k, (e, dh) in enumerate([(e, dh) for e in range(4 * g, 4 * g + 4)
                                     for dh in range(DH)]):
            mm = nc.tensor.matmul(
                tp[:, k, :], q_f32[:, e, dh * 128:(dh + 1) * 128], id_f32[:, :],
                is_transpose=True,
                start=(first is None), stop=(k == 7), skip_group_check=True,
            )
            if first is None:
                first = mm
            else:
                add_dep_helper(mm.ins, first.ins, sync=False, reason="bank order")
        psum_copy(
            qT_sb[:, 4 * g:4 * g + 4, :, :].rearrange("p e dh j -> p (e dh) j"),
            tp[:, :, :])
    for dh in range(DH):
        nc.vector.tensor_scalar(
            rhs_qm[:, :, dh, :], qT_sb[:, :, dh, :],
            w_sb[:, 4 + dh:5 + dh], w_sb[:, 0 + dh:1 + dh],
            op0=OP.mult, op1=OP.add,
        )
    ps_qw = ps_misc_pool.tile([128, 512], F32, tag="misc")
    for dh in range(DH):
        nc.tensor.matmul(
            ps_qw[0:1, 0:EX * Q].rearrange("o (e j) -> o e j", e=EX),
            wq_bf[:, dh:dh + 1],
            qT_sb[:, :, dh, :],
            start=(dh == 0), stop=(dh == DH - 1),
        )
    nc.scalar.copy(qwrow[0:1, :, :], ps_qw[0:1, 0:EX * Q].rearrange("o (e j) -> o e j", e=EX))

    # ---------- per-pair pipeline, stage-skewed ----------
    # Engine queues execute in program order, so pair p's late stages must
    # not sit ahead of pair p+1's independent early stages.  Emit rounds:
    # A(0); A(1) D(0); A(2) D(1) E(0); A(3) D(2) E(1); D(3) E(2); E(3).

    def stage_A(pair):
        """c^T transposes+evac, S matmuls, exp."""
        e0, e1 = 2 * pair, 2 * pair + 1
        eP = slice(e0, e1 + 1)
        for e in (e0, e1):
            for dh in range(DH):
                tp2 = ps_tp_pool.tile([128, CH, 128], BF16, tag="tp")
                pe_transpose_group(
                    tp2,
                    [c_nat[:, e, chk, dh * 128:(dh + 1) * 128] for chk in range(CH)],
                )
                psum_copy(c_T[:, e, dh, :], tp2[:, :, :])

        ps_s = ps_s_pool.tile([128, CH, 2, Q], F32)
        first_mm = None
        for slot in range(2):
            e = e0 + slot
            for chk in range(CH):
                for dh in range(DH):
                    mm = nc.tensor.matmul(
                        ps_s[:, chk, slot, :],
                        c_T[:, e, dh, chk * 128:(chk + 1) * 128],
                        rhs_qm[:, e, dh, :],
                        start=(first_mm is None), stop=False,
                        skip_group_check=True,
                    )
                    if first_mm is None:
                        first_mm = mm
                    else:
                        add_dep_helper(mm.ins, first_mm.ins, sync=False,
                                       reason="bank clear order")
            mm = nc.tensor.matmul(
                ps_s[:, :, slot, :],
                ones_bf[0:1, :],
                qwrow[0:1, e, :].unsqueeze(1).broadcast_to([1, CH, Q]),
                start=False, stop=(slot == 1),
                skip_group_check=True,
            )
            add_dep_helper(mm.ins, first_mm.ins, sync=False, reason="bank clear order")

        nc.scalar.activation(P_all[:, :, eP, :], ps_s[:, :, :, :], AF.Exp)

    def stage_D(pair):
        """Row sums + maxes, reciprocal, Pn (all DVE)."""
        e0, e1 = 2 * pair, 2 * pair + 1
        eP = slice(e0, e1 + 1)
        pview = P_all[:, :, eP, :]
        den = den_pool.tile([128, CH, 2], F32)
        nc.vector.reduce_sum(den[:, :, :], pview, axis=AX.X)
        nc.vector.tensor_reduce(pm_col[:, :, eP], pview, axis=AX.X, op=OP.max)
        rden = den_pool.tile([128, CH, 2], F32, tag="rden")
        nc.vector.reciprocal(rden[:, :, :], den[:, :, :])
        nc.vector.tensor_tensor(
            Pn_all[:, :, eP, :], pview,
            rden[:, :, :].unsqueeze(3).broadcast_to([128, CH, 2, Q]),
            op=OP.mult,
        )

    def stage_E(pair):
        """Pn^T, b_att norm, c2q, folds, q2c, piece3."""
        e0, e1 = 2 * pair, 2 * pair + 1
        eP = slice(e0, e1 + 1)

        tp = ps_tp_pool.tile([128, CH, 128], BF16, tag="tp")
        pe_transpose_group(tp, [Pn_all[:, chk, eP, :] for chk in range(CH)])
        psum_copy(PT_all[:, pair, :, :], tp[:, :, :])

        # b_att normalizers + q2c (PE-light, pulls work off the tail)
        ps_pair = ps_misc_pool.tile([128, 512], F32, tag="misc")
        nc.tensor.matmul(
            ps_pair[0:1, 0:2 * CH].rearrange("o (e c) -> o e c", c=CH),
            ones128_bf[:, :],
            pm_col[:, :, eP].rearrange("p c e -> p e c"),
            start=True, stop=True,
        )
        nc.vector.reduce_sum(
            sumb[0:1, eP],
            ps_pair[0:1, 0:2 * CH].rearrange("o (e c) -> o e c", c=CH),
            axis=AX.X,
        )
        nc.vector.reciprocal(recipb[0:1, eP], sumb[0:1, eP])
        nc.tensor.matmul(
            ps_pair[:, 8:10], ones_f32[0:1, :], recipb[0:1, eP],
            start=True, stop=True,
        )
        nc.scalar.copy(r_sb[:, eP], ps_pair[:, 8:10])

        for dh in range(DH):
            for slot in range(2):
                e = e0 + slot
                ps_c2q = ps_c2q_pool.tile([128, C], F32)
                nc.tensor.matmul(
                    ps_c2q[:, :],
                    q_dup[slot * 64:slot * 64 + 64, e, dh * 128:(dh + 1) * 128],
                    PT_all[slot * 64:slot * 64 + 64, pair, :, :],
                    start=True, stop=True,
                    tile_position=(slot * 64, 0),
                )
                psum_copy(c2q_sb[:, e, dh, :], ps_c2q[:, :])

        cT_p = c_T[:, eP, :, :]                    # [128, 2, DH, C]
        c2_p = c2q_sb[:, eP, :, :]
        prod = scr_pool.tile([128, 2, DH, C], BF16, tag="prod")
        # per-dh split so folding starts after half the c2q evacuations
        stk1 = scr_pool.tile([128, 3, 2, DH, 256], BF16, tag="stk1")
        for dh in range(DH):
            nc.vector.tensor_tensor(
                prod[:, :, dh, :], cT_p[:, :, dh, :], c2_p[:, :, dh, :], op=OP.mult)
            nc.vector.tensor_tensor(
                stk1[:, 0, :, dh, :], c2_p[:, :, dh, 0:256], c2_p[:, :, dh, 256:512], op=OP.max)
            nc.vector.tensor_tensor(
                stk1[:, 1, :, dh, :], prod[:, :, dh, 0:256], prod[:, :, dh, 256:512], op=OP.max)
            nc.vector.tensor_tensor(
                stk1[:, 2, :, dh, :], cT_p[:, :, dh, 0:256], cT_p[:, :, dh, 256:512], op=OP.max)
        stk2 = scr_pool.tile([128, 3, 2, DH, 128], BF16, tag="stk2")
        nc.vector.tensor_tensor(
            stk2[:, :, :, :, :], stk1[:, :, :, :, 0:128], stk1[:, :, :, :, 128:256], op=OP.max)
        stk3 = scr_pool.tile([128, 3, 2, DH, 64], BF16, tag="stk3")
        nc.vector.tensor_tensor(
            stk3[:, :, :, :, :], stk2[:, :, :, :, 0:64], stk2[:, :, :, :, 64:128], op=OP.max)
        stk4 = scr_pool.tile([128, 3, 2, DH, 32], BF16, tag="stk4")
        nc.vector.tensor_tensor(
            stk4[:, :, :, :, :], stk3[:, :, :, :, 0:32], stk3[:, :, :, :, 32:64], op=OP.max)
        nc.vector.tensor_reduce(
            fview[:, 0:3, :, eP].rearrange("p pc dh e -> p pc e dh"),
            stk4[:, :, :, :, :], axis=AX.X, op=OP.max)

        f1 = scr_pool.tile([128, 2, DH, 256], BF16, tag="bigA")
        nc.vector.tensor_tensor(
            f1[:, :, :, :], cT_p[:, :, :, 0:256], cT_p[:, :, :, 256:512], op=OP.min)
        f2 = scr_pool.tile([128, 2, DH, 128], BF16, tag="bigB")
        nc.vector.tensor_tensor(
            f2[:, :, :, :], f1[:, :, :, 0:128], f1[:, :, :, 128:256], op=OP.min)
        f3 = scr_pool.tile([128, 2, DH, 64], BF16, tag="bigC")
        nc.vector.tensor_tensor(
            f3[:, :, :, :], f2[:, :, :, 0:64], f2[:, :, :, 64:128], op=OP.min)
        nc.vector.tensor_reduce(
            cminv[:, :, eP].rearrange("p dh e -> p e dh"),
            f3[:, :, :, :], axis=AX.X, op=OP.min)

        for slot in range(2):
            e = e0 + slot
            ps_m2 = ps_misc_pool.tile([128, 512], F32, tag="misc")
            for chk in range(CH):
                nc.tensor.matmul(
                    ps_m2[0:1, 64:64 + H2],
                    pm_col[:, chk, e:e + 1],
                    c_nat[:, e, chk, :],
                    start=(chk == 0), stop=(chk == CH - 1),
                )
            q2c_sc = q2_pool.tile([1, H2], F32)
            nc.scalar.mul(q2c_sc[0:1, :], ps_m2[0:1, 64:64 + H2], r_sb[0:1, e:e + 1])
            for dh in range(DH):
                nc.tensor.matmul(
                    ps_m2[:, 320 + dh:321 + dh],
                    q2c_sc[0:1, dh * 128:(dh + 1) * 128],
                    ones_f32[0:1, 0:1],
                    is_transpose=True,
                    start=(dh == 0), stop=(dh == DH - 1),
                    skip_group_check=True,
                )
            nc.scalar.copy(q2cT_sb[:, e, :], ps_m2[:, 320:322])

        s3a = scr_pool.tile([128, 2, DH], F32, tag="s3a")
        s3b = scr_pool.tile([128, 2, DH], F32, tag="s3b")
        nc.vector.tensor_tensor(
            s3a[:, :, :], q2cT_sb[:, eP, :],
            fview[:, 2, :, eP].rearrange("p dh e -> p e dh"), op=OP.mult)
        nc.vector.tensor_tensor(
            s3b[:, :, :], q2cT_sb[:, eP, :],
            cminv[:, :, eP].rearrange("p dh e -> p e dh"), op=OP.mult)
        nc.vector.tensor_tensor(
            fview[:, 3, :, eP].rearrange("p dh e -> p e dh"),
            s3a[:, :, :], s3b[:, :, :], op=OP.max)

    stage_A(0)
    stage_A(1); stage_D(0)
    stage_A(2); stage_D(1); stage_E(0)
    stage_A(3); stage_D(2); stage_E(1)
    stage_D(3); stage_E(2)
    stage_E(3)

    # ---------- final: out = max-pooled features @ w_label + b ----------
    ps_out = ps_misc_pool.tile([128, 512], F32, tag="misc")
    REF_PC = (1, 2, 0, 3)
    for k in range(NK):
        pc, dh = k // DH, k % DH
        nc.tensor.matmul(
            ps_out[0:EX, 0:NL], final_f[:, k * EX:(k + 1) * EX],
            wlab_sb[:, REF_PC[pc] * DH + dh, :],
            start=(k == 0), stop=False, skip_group_check=True,
        )
    nc.tensor.matmul(
        ps_out[0:EX, 0:NL], ones_f32[0:1, 0:EX], b_sb[0:1, :],
        start=False, stop=True, skip_group_check=True,
    )
    nc.scalar.copy(out_sb[:, :], ps_out[0:EX, 0:NL])
    nc.sync.dma_start(out[:, :], out_sb[:, :])


def build_nc():
    nc = bacc.Bacc("TRN2", target_bir_lowering=False, debug=False)
    fd = nc.dram_tensor("fd", [EX, C, H2], F32, kind="ExternalInput")
    fq = nc.dram_tensor("fq", [EX, Q, H2], F32, kind="ExternalInput")
    wsim = nc.dram_tensor("wsim", [3 * H2], F32, kind="ExternalInput")
    wlab = nc.dram_tensor("wlab", [4 * H2, NL], F32, kind="ExternalInput")
    blab = nc.dram_tensor("blab", [NL], F32, kind="ExternalInput")
    out = nc.dram_tensor("out", [EX, NL], F32, kind="ExternalOutput")

    from contextlib import ExitStack
    with tile.TileContext(nc) as tc:
        with ExitStack() as ctx:
            _body(tc, ctx, fd[:, :, :], fq[:, :, :], wsim[:], wlab[:, :], blab[:], out[:, :])
    nc.compile()
    return nc


_NC_CACHE = None


def run(inputs, trace=False):
    global _NC_CACHE
    if _NC_CACHE is None:
        _NC_CACHE = build_nc()
    nc = _NC_CACHE

    fd = np.ascontiguousarray(np.asarray(inputs["feature_document"], dtype=np.float32))
    fq = np.ascontiguousarray(np.asarray(inputs["feature_query"], dtype=np.float32))
    wsim = np.ascontiguousarray(np.asarray(inputs["w_sim"], dtype=np.float32))
    wlab = np.ascontiguousarray(np.asarray(inputs["w_label"], dtype=np.float32))
    blab = np.ascontiguousarray(np.asarray(inputs["b_label"], dtype=np.float32))

    in_maps = []
    for core in range(N_CORES):
        sl = slice(core * EX, (core + 1) * EX)
        in_maps.append({
            "fd": fd[sl], "fq": fq[sl],
            "wsim": wsim, "wlab": wlab, "blab": blab,
        })
    res = run_bass_kernel_spmd(nc, in_maps, list(range(N_CORES)), trace=trace)
    outs = np.concatenate([np.asarray(res.results[i]["out"]) for i in range(N_CORES)], axis=0)
    return outs.astype(np.float32), res


def kernel(**inputs):
    outs, _ = run(inputs, trace=False)
    return outs
